# revision 1
# baseline (speedup 1.0000x reference)
"""Trainium2 Bass kernel for skipgram-style edge loss (embedding_lookup).

reference:
    u = emb[pos[:,0]]; v = emb[pos[:,1]]
    nu = emb[neg[...,0]]; nv = emb[neg[...,1]]
    loss = softplus(-<u,v>) + sum_k softplus(<nu_k,nv_k>)      # [E]

Strategy: replicate the 256MB table into each core's DRAM, split the 50k
edge batch 8 ways.  Each core performs row gathers via SWDGE indirect DMA
(one 512B f32 descriptor per embedding row; bf16 cast-on-gather measured
SLOWER because 256B SBUF writes drop the SDMA engines below line rate).
DVE does elementwise mul + segmented reduce per ~1MB chunk so it pipelines
against the SWDGE queue; ACT applies softplus with the pos-edge sign flip
folded into the activation scale.

Task layout per core: edge e_local = (t*128 + p)*M + i maps to device
tile t, partition p, inner slot i; task j (0=pos, 1..5=neg) is the OUTER
slot dim (slot = j*M + i), so the pos/neg sign split is two contiguous
column ranges.
"""

import numpy as np

import concourse.bacc as bacc
import concourse.bass as bass
import concourse.mybir as mybir
from concourse.tile import TileContext
from concourse.bass_utils import run_bass_kernel_spmd

# Problem sizes (hardcoded per contract)
V = 500_000
D = 128
E = 50_000
K = 5

NCORES = 8
P = 128
J = K + 1                      # dot products per edge (1 pos + K neg)
EPC = E // NCORES              # 6250 edges per core
M = 7                          # edges per partition per tile
NT = -(-EPC // (P * M))        # 7 tiles per core
EPAD = NT * P * M              # 6272 padded edges per core
KSLOT = M * J                  # 42 dot slots per partition per tile

LAST_RESULTS = None            # BassKernelResults of the most recent run


def build_program(v=V, d=D, nt=NT, m=M, j=J, native_softplus=False):
    """native_softplus=True is unavailable: walrus has no ACT table entry for
    Softplus on this build ("no activation table contains Some(Softplus)").
    The default path computes softplus via exp + product tree + one final ln:
    sum_j ln(1+e^x_j) = ln prod_j (1+e^x_j)."""
    kslot = m * j
    nc = bacc.Bacc(trn_type="TRN2")
    emb = nc.dram_tensor("embeddings", [v, d], mybir.dt.float32,
                         kind="ExternalInput")
    # [:, :nt*kslot] = left rows, [:, nt*kslot:] = right rows
    idx = nc.dram_tensor("idx", [P, 2 * nt * kslot], mybir.dt.int32,
                         kind="ExternalInput")
    loss = nc.dram_tensor("loss", [P, nt * m], mybir.dt.float32,
                          kind="ExternalOutput")

    with TileContext(nc) as tc:
        with (
            tc.tile_pool(name="io", bufs=1) as io_pool,
            tc.tile_pool(name="emb", bufs=6) as emb_pool,
            tc.tile_pool(name="small", bufs=3) as small_pool,
        ):
            loss_sb = io_pool.tile([P, nt * m], mybir.dt.float32)

            # single idx load: per-tile idx DMAs measured SLOWER (the small
            # 168B-descriptor HWDGE packets round-robin with the gather rings
            # and drop SWDGE streaming efficiency by ~20%)
            idx_sb = io_pool.tile([P, 2 * nt * kslot], mybir.dt.int32)
            nc.sync.dma_start(idx_sb[:], idx[:])
            idxl_sb = [idx_sb[:, t * kslot:(t + 1) * kslot]
                       for t in range(nt)]
            idxr_sb = [idx_sb[:, (nt + t) * kslot:(nt + t + 1) * kslot]
                       for t in range(nt)]

            # chunking: split each tile's kslot columns so gather->mul->reduce
            # pipelines at sub-tile granularity and the final tile's
            # critical-path tail is one chunk, not a whole tile
            nchunks = 3 if kslot % 3 == 0 else 1
            csl = kslot // nchunks

            for t in range(nt):
                dots = small_pool.tile([P, kslot], mybir.dt.float32, tag="dots")
                for c in range(nchunks):
                    lo = c * csl
                    el = emb_pool.tile([P, csl * d], mybir.dt.float32,
                                       tag="el")
                    er = emb_pool.tile([P, csl * d], mybir.dt.float32,
                                       tag="er")
                    nc.gpsimd.indirect_dma_start(
                        out=el[:], out_offset=None, in_=emb[:],
                        in_offset=bass.IndirectOffsetOnAxis(
                            ap=idxl_sb[t][:, lo:lo + csl], axis=0))
                    nc.gpsimd.indirect_dma_start(
                        out=er[:], out_offset=None, in_=emb[:],
                        in_offset=bass.IndirectOffsetOnAxis(
                            ap=idxr_sb[t][:, lo:lo + csl], axis=0))
                    nc.vector.tensor_mul(el[:], el[:], er[:])
                    nc.vector.reduce_sum(
                        dots[:, lo:lo + csl],
                        el[:].rearrange("p (k d) -> p k d", d=d),
                        axis=mybir.AxisListType.X)

                if native_softplus:
                    # softplus(s_j * dot): j=0 slots (positive edges) scale -1
                    sp = small_pool.tile([P, kslot], mybir.dt.float32,
                                         tag="sp")
                    nc.scalar.activation(
                        sp[:, :m], dots[:, :m],
                        mybir.ActivationFunctionType.Softplus, scale=-1.0)
                    nc.scalar.activation(
                        sp[:, m:], dots[:, m:],
                        mybir.ActivationFunctionType.Softplus, scale=1.0)
                    # loss = sum over the J tasks of each edge (stride-m cols)
                    nc.vector.reduce_sum(
                        loss_sb[:, t * m:(t + 1) * m],
                        sp[:].rearrange("p (j i) -> p i j", i=m),
                        axis=mybir.AxisListType.X)
                else:
                    # ln(prod_j (1 + exp(s_j dot_j))) via exp + product tree
                    ex = small_pool.tile([P, kslot], mybir.dt.float32,
                                         tag="ex")
                    nc.scalar.activation(ex[:, :m], dots[:, :m],
                                         mybir.ActivationFunctionType.Exp,
                                         scale=-1.0)
                    nc.scalar.activation(ex[:, m:], dots[:, m:],
                                         mybir.ActivationFunctionType.Exp,
                                         scale=1.0)
                    nc.vector.tensor_scalar_add(ex[:], ex[:], 1.0)
                    assert j == 6
                    b = small_pool.tile([P, 3 * m], mybir.dt.float32, tag="b")
                    cc = small_pool.tile([P, m], mybir.dt.float32, tag="c")
                    nc.vector.tensor_mul(b[:], ex[:, :3 * m], ex[:, 3 * m:])
                    nc.vector.tensor_mul(cc[:], b[:, :m], b[:, m:2 * m])
                    nc.vector.tensor_mul(loss_sb[:, t * m:(t + 1) * m],
                                         cc[:], b[:, 2 * m:])

            if not native_softplus:
                nc.scalar.activation(loss_sb[:], loss_sb[:],
                                     mybir.ActivationFunctionType.Ln)
            nc.sync.dma_start(loss[:], loss_sb[:])
    nc.finalize()
    return nc


def _pack_indices(pos_edges, neg_edges, core):
    """Build the [P, 2*NT*KSLOT] int32 row-index array for one core."""
    lo = core * EPC
    hi = lo + EPC
    tl = np.zeros((EPAD, J), np.int32)
    tr = np.zeros((EPAD, J), np.int32)
    tl[:EPC, 0] = pos_edges[lo:hi, 0]
    tl[:EPC, 1:] = neg_edges[lo:hi, :, 0]
    tr[:EPC, 0] = pos_edges[lo:hi, 1]
    tr[:EPC, 1:] = neg_edges[lo:hi, :, 1]
    # [EPAD, J] -> [NT, P, M, J] -> [P, NT, J, M] -> [P, NT*KSLOT]
    il = tl.reshape(NT, P, M, J).transpose(1, 0, 3, 2).reshape(P, NT * KSLOT)
    ir = tr.reshape(NT, P, M, J).transpose(1, 0, 3, 2).reshape(P, NT * KSLOT)
    return np.ascontiguousarray(np.concatenate([il, ir], axis=1))


_PROGRAM = None


def kernel(embeddings, pos_edges, neg_edges):
    global _PROGRAM, LAST_RESULTS
    embeddings = np.ascontiguousarray(np.asarray(embeddings, dtype=np.float32))
    pos_edges = np.asarray(pos_edges).astype(np.int32)
    neg_edges = np.asarray(neg_edges).astype(np.int32)

    if _PROGRAM is None:
        _PROGRAM = build_program()
    nc = _PROGRAM

    in_maps = [
        {"embeddings": embeddings,
         "idx": _pack_indices(pos_edges, neg_edges, c)}
        for c in range(NCORES)
    ]

    res = run_bass_kernel_spmd(nc, in_maps, core_ids=list(range(NCORES)))
    LAST_RESULTS = res

    out = np.empty(E, np.float32)
    for c in range(NCORES):
        dev = np.asarray(res.results[c]["loss"], np.float32)  # [P, NT*M]
        ordered = dev.reshape(P, NT, M).transpose(1, 0, 2).reshape(EPAD)
        out[c * EPC:(c + 1) * EPC] = ordered[:EPC]
    return out



# revision 2
# speedup vs baseline: 1.5890x; 1.5890x over previous
"""Trainium2 Bass kernel for skipgram-style edge loss (embedding_lookup).

reference:
    u = emb[pos[:,0]]; v = emb[pos[:,1]]
    nu = emb[neg[...,0]]; nv = emb[neg[...,1]]
    loss = softplus(-<u,v>) + sum_k softplus(<nu_k,nv_k>)      # [E]

Strategy: replicate the table into each core's DRAM as bf16 (tolerance is
2e-2; bf16 quantization error on the loss is ~1e-6), split the 50k edge
batch 8 ways.  Each core performs row gathers via SWDGE indirect DMA
(one 256B bf16 descriptor per embedding row - half the HBM traffic and
half the per-descriptor SDMA beat count vs f32).  DVE does the pairwise
mul at bf16 2x mode, then reduces d=128 with two halving tensor_tensor
adds (2x mode) plus one short 1x tensor_reduce - faster than a single
full-width tensor_reduce, which has no 2x uop.  ACT applies exp with the
pos-edge sign flip folded into the activation scale; softplus is computed
as ln(prod_j(1+e^x_j)) with one final Ln.

Task layout per core: edge e_local = (t*128 + p)*M + i maps to device
tile t, partition p, inner slot i; task j (0=pos, 1..5=neg) is the OUTER
slot dim (slot = j*M + i), so the pos/neg sign split is two contiguous
column ranges.
"""

import ml_dtypes
import numpy as np

import concourse.bacc as bacc
import concourse.bass as bass
import concourse.mybir as mybir
from concourse.tile import TileContext
from concourse.bass_utils import run_bass_kernel_spmd

# Problem sizes (hardcoded per contract)
V = 500_000
D = 128
E = 50_000
K = 5

NCORES = 8
P = 128
J = K + 1                      # dot products per edge (1 pos + K neg)
EPC = E // NCORES              # 6250 edges per core
M = 7                          # edges per partition per tile
NT = -(-EPC // (P * M))        # 7 tiles per core
EPAD = NT * P * M              # 6272 padded edges per core
KSLOT = M * J                  # 42 dot slots per partition per tile

LAST_RESULTS = None            # BassKernelResults of the most recent run


def build_program(v=V, d=D, nt=NT, m=M, j=J, nchunks=3, emb_bufs=8):
    """softplus via exp + product tree + one final ln:
    sum_j ln(1+e^x_j) = ln prod_j (1+e^x_j).  (walrus has no ACT table
    entry for native Softplus on this build.)"""
    kslot = m * j
    nc = bacc.Bacc(trn_type="TRN2")
    emb = nc.dram_tensor("embeddings", [v, d], mybir.dt.bfloat16,
                         kind="ExternalInput")
    # [:, :nt*kslot] = left rows, [:, nt*kslot:] = right rows
    idx = nc.dram_tensor("idx", [P, 2 * nt * kslot], mybir.dt.int32,
                         kind="ExternalInput")
    loss = nc.dram_tensor("loss", [P, nt * m], mybir.dt.float32,
                          kind="ExternalOutput")

    with TileContext(nc) as tc:
        with (
            tc.tile_pool(name="io", bufs=1) as io_pool,
            tc.tile_pool(name="emb", bufs=emb_bufs) as emb_pool,
            tc.tile_pool(name="small", bufs=3) as small_pool,
        ):
            loss_sb = io_pool.tile([P, nt * m], mybir.dt.float32)

            # single idx load: per-tile idx DMAs measured SLOWER (the small
            # 168B-descriptor HWDGE packets round-robin with the gather rings
            # and drop SWDGE streaming efficiency by ~20%)
            idx_sb = io_pool.tile([P, 2 * nt * kslot], mybir.dt.int32)
            nc.sync.dma_start(idx_sb[:], idx[:])
            idxl_sb = [idx_sb[:, t * kslot:(t + 1) * kslot]
                       for t in range(nt)]
            idxr_sb = [idx_sb[:, (nt + t) * kslot:(nt + t + 1) * kslot]
                       for t in range(nt)]

            assert kslot % nchunks == 0
            csl = kslot // nchunks

            for t in range(nt):
                dots = small_pool.tile([P, kslot], mybir.dt.float32, tag="dots")
                for c in range(nchunks):
                    lo = c * csl
                    el = emb_pool.tile([P, csl * d], mybir.dt.bfloat16,
                                       tag="el")
                    er = emb_pool.tile([P, csl * d], mybir.dt.bfloat16,
                                       tag="er")
                    nc.gpsimd.indirect_dma_start(
                        out=el[:], out_offset=None, in_=emb[:],
                        in_offset=bass.IndirectOffsetOnAxis(
                            ap=idxl_sb[t][:, lo:lo + csl], axis=0))
                    nc.gpsimd.indirect_dma_start(
                        out=er[:], out_offset=None, in_=emb[:],
                        in_offset=bass.IndirectOffsetOnAxis(
                            ap=idxr_sb[t][:, lo:lo + csl], axis=0))
                    # pairwise mul at bf16 2x mode, in place
                    nc.vector.tensor_mul(el[:], el[:], er[:])
                    # d=128 -> 64 -> 32 via 2x-mode adds, then 1x reduce(32)
                    h1 = small_pool.tile([P, csl * (d // 2)],
                                         mybir.dt.bfloat16, tag="h1")
                    pv = el[:].rearrange("p (k two h) -> p k two h",
                                         two=2, h=d // 2)
                    nc.vector.tensor_add(h1[:], pv[:, :, 0, :], pv[:, :, 1, :])
                    h2 = small_pool.tile([P, csl * (d // 4)],
                                         mybir.dt.bfloat16, tag="h2")
                    hv = h1[:].rearrange("p (k two h) -> p k two h",
                                         two=2, h=d // 4)
                    nc.vector.tensor_add(h2[:], hv[:, :, 0, :], hv[:, :, 1, :])
                    nc.vector.reduce_sum(
                        dots[:, lo:lo + csl],
                        h2[:].rearrange("p (k h) -> p k h", h=d // 4),
                        axis=mybir.AxisListType.X)

                # ln(prod_j (1 + exp(s_j dot_j))) via exp + product tree
                ex = small_pool.tile([P, kslot], mybir.dt.float32,
                                     tag="ex")
                nc.scalar.activation(ex[:, :m], dots[:, :m],
                                     mybir.ActivationFunctionType.Exp,
                                     scale=-1.0)
                nc.scalar.activation(ex[:, m:], dots[:, m:],
                                     mybir.ActivationFunctionType.Exp,
                                     scale=1.0)
                nc.vector.tensor_scalar_add(ex[:], ex[:], 1.0)
                assert j == 6
                b = small_pool.tile([P, 3 * m], mybir.dt.float32, tag="b")
                cc = small_pool.tile([P, m], mybir.dt.float32, tag="c")
                nc.vector.tensor_mul(b[:], ex[:, :3 * m], ex[:, 3 * m:])
                nc.vector.tensor_mul(cc[:], b[:, :m], b[:, m:2 * m])
                nc.vector.tensor_mul(loss_sb[:, t * m:(t + 1) * m],
                                     cc[:], b[:, 2 * m:])

            nc.scalar.activation(loss_sb[:], loss_sb[:],
                                 mybir.ActivationFunctionType.Ln)
            nc.sync.dma_start(loss[:], loss_sb[:])
    nc.finalize()
    return nc


def _pack_indices(pos_edges, neg_edges, core):
    """Build the [P, 2*NT*KSLOT] int32 row-index array for one core."""
    lo = core * EPC
    hi = lo + EPC
    tl = np.zeros((EPAD, J), np.int32)
    tr = np.zeros((EPAD, J), np.int32)
    tl[:EPC, 0] = pos_edges[lo:hi, 0]
    tl[:EPC, 1:] = neg_edges[lo:hi, :, 0]
    tr[:EPC, 0] = pos_edges[lo:hi, 1]
    tr[:EPC, 1:] = neg_edges[lo:hi, :, 1]
    # [EPAD, J] -> [NT, P, M, J] -> [P, NT, J, M] -> [P, NT*KSLOT]
    il = tl.reshape(NT, P, M, J).transpose(1, 0, 3, 2).reshape(P, NT * KSLOT)
    ir = tr.reshape(NT, P, M, J).transpose(1, 0, 3, 2).reshape(P, NT * KSLOT)
    return np.ascontiguousarray(np.concatenate([il, ir], axis=1))


_PROGRAM = None


def kernel(embeddings, pos_edges, neg_edges):
    global _PROGRAM, LAST_RESULTS
    emb_bf16 = np.ascontiguousarray(
        np.asarray(embeddings, dtype=np.float32).astype(ml_dtypes.bfloat16))
    pos_edges = np.asarray(pos_edges).astype(np.int32)
    neg_edges = np.asarray(neg_edges).astype(np.int32)

    if _PROGRAM is None:
        _PROGRAM = build_program()
    nc = _PROGRAM

    in_maps = [
        {"embeddings": emb_bf16,
         "idx": _pack_indices(pos_edges, neg_edges, c)}
        for c in range(NCORES)
    ]

    res = run_bass_kernel_spmd(nc, in_maps, core_ids=list(range(NCORES)))
    LAST_RESULTS = res

    out = np.empty(E, np.float32)
    for c in range(NCORES):
        dev = np.asarray(res.results[c]["loss"], np.float32)  # [P, NT*M]
        ordered = dev.reshape(P, NT, M).transpose(1, 0, 2).reshape(EPAD)
        out[c * EPC:(c + 1) * EPC] = ordered[:EPC]
    return out
